# revision 1
# baseline (speedup 1.0000x reference)
"""Trainium2 Bass kernel for nn_CrossHeadAttention.

Computation (per batch b):
  pooled = mean(x[b], spatial)                       # (NH, CH)
  aw     = tiny transformer block on pooled          # (NH, CH)
  out[b] = x[b] * (1 + aw)[..., None, None]

Memory-bound: 256 MiB in + 256 MiB out. Sharding: pure data-parallel over
batch (32 batches -> 8 cores x 4 batches). Per core, each batch's
(4, 8, 256, 256) slab is viewed as a [128, 16384] SBUF tile
(partition = head*32 + ch*4 + spatial_quarter), streamed chunk-wise:
load -> DVE spatial reduce -> tiny PE/DVE/ACT attention math ->
ACT broadcast multiply (in place) -> store.

The four batches' tiny-math chains are long serial dependency chains
(~45 ops each). They are emitted one after another (each chain is
data-bound on its own batch's loads, so interleaving ops from a younger,
still-loading batch would only head-of-line block the in-order engines),
and each batch's four broadcast multiplies are split 2-on-ACT/2-on-DVE
so the store stream starts sooner.
"""

from contextlib import ExitStack

import numpy as np

import concourse.bacc as bacc
import concourse.bass as bass
import concourse.tile as tile
from concourse import mybir

NCORES = 8
B, NH, CH = 32, 4, 8
H = W = 256
S = H * W                  # spatial elements per (b, h, c) plane
HID = 4
BPC = B // NCORES          # batches per core
P = 128                    # SBUF partitions
SPLIT = P // (NH * CH)     # spatial quarters mapped to partitions
FREE = S // SPLIT          # free-dim elements per partition
NCHUNK = 8
SCALE = CH ** -0.5
EPS = 1e-5
F32 = mybir.dt.float32
AFT = mybir.ActivationFunctionType
ALU = mybir.AluOpType
AX = mybir.AxisListType

# CoreSim has no Gelu; sim checks can swap this for an implemented function
_GELU = AFT.Gelu
_RSTD_LNEXP = False
_HEAD_START = 999   # chains run serially: each is unstalled and data-bound
_M4_FOLD = True
_XBUFS = 12
_MULT_SPLIT = True  # 2 multiplies on ACT + 2 on DVE per batch


def _emit(nc, tc, io):
    with ExitStack() as ctx:
        const = ctx.enter_context(tc.tile_pool(name="const", bufs=1))
        xp = ctx.enter_context(tc.tile_pool(name="xp", bufs=_XBUFS * NCHUNK // 4))
        sm = ctx.enter_context(tc.tile_pool(name="sm", bufs=6))
        ps = ctx.enter_context(tc.tile_pool(name="ps", bufs=8, space="PSUM"))

        def ld_mat(name, p, f):
            t = const.tile([p, f], F32, tag="c_" + name)
            nc.gpsimd.dma_start(out=t, in_=io[name][:])
            return t

        def ld_bcast(name, f, parts=NH):
            # DRAM vector [f] -> SBUF [parts, f], replicated across partitions
            t = const.tile([parts, f], F32, tag="cb_" + name)
            hap = io[name][:]
            src = bass.AP(tensor=hap.tensor, offset=hap.offset,
                          ap=[[0, parts]] + list(hap.ap))
            nc.gpsimd.dma_start(out=t, in_=src)
            return t

        wq_t = ld_mat("wq_t", CH, CH)
        wk_t = ld_mat("wk_t", CH, CH)
        wv_t = ld_mat("wv_t", CH, CH)
        wo_t = ld_mat("wo_t", CH, CH)
        w1_t = ld_mat("w1_t", CH, HID)
        w2_t = ld_mat("w2_t", HID, CH)
        eye4 = ld_mat("eye4", NH, NH)
        bo_bc = ld_bcast("bo", CH)
        b1_bc = ld_bcast("b1", HID)
        b2_bc = ld_bcast("b2", CH)
        g1_bc = ld_bcast("g1", CH)
        beta1_bc = ld_bcast("beta1", CH)
        g2_bc = ld_bcast("g2", CH)
        beta2_bc = ld_bcast("beta2", CH)

        # selection constants for cross-partition moves via PE matmul
        # (partition k of an x tile holds (h, c, q) = (k//32, (k%32)//4, k%4))
        cmask = ld_mat("cmask", P, CH)     # [k, c] = (c(k)==c) / S
        hsel = ld_mat("hsel", P, NH)       # [k, h] = (h(k)==h)
        b128 = ld_mat("b128", CH, P)       # [c, k] = (c(k)==c)
        ind128 = ld_mat("ind128", NH, P)   # [h, k] = (h(k)==h)
        ones4 = const.tile([NH, 1], F32, tag="c_ones4")
        nc.vector.memset(ones4, 1.0)

        eps4 = const.tile([NH, 1], F32, tag="c_eps4")
        nc.vector.memset(eps4, EPS)
        graw = ld_bcast("gate", 1)
        gsig4 = const.tile([NH, 1], F32, tag="c_gsig4")
        nc.scalar.activation(out=gsig4, in_=graw, func=AFT.Sigmoid)
        omg4 = const.tile([NH, 1], F32, tag="c_omg4")      # 1 - sigmoid(gate)
        nc.vector.tensor_scalar(out=omg4, in0=gsig4, scalar1=-1.0, scalar2=1.0,
                                op0=ALU.mult, op1=ALU.add)

        def pe_t(src, f, tag):
            # [4, f] -> [f, 4] via PE transpose (fp32 has no DMA transpose)
            tp = ps.tile([f, NH], F32, tag="ps")
            nc.tensor.transpose(tp, src, eye4)
            t = sm.tile([f, NH], F32, tag=tag)
            nc.vector.tensor_copy(out=t, in_=tp)
            return t

        def mm(lhsT, rhs, m, n, tag=None):
            op = ps.tile([m, n], F32, tag="ps")
            nc.tensor.matmul(op, lhsT, rhs, start=True, stop=True)
            if tag is None:
                return op
            t = sm.tile([m, n], F32, tag=tag)
            nc.vector.tensor_copy(out=t, in_=op)
            return t

        def layernorm(src, g_bc, b_bc, tag):
            stats = sm.tile([NH, nc.vector.BN_STATS_DIM], F32, tag=tag + "_st")
            nc.vector.bn_stats(out=stats, in_=src)
            mv = sm.tile([NH, 2], F32, tag=tag + "_mv")
            nc.vector.bn_aggr(out=mv, in_=stats)
            yield
            if _RSTD_LNEXP:
                # rstd = exp(-0.5 * ln(var + eps)): keeps ACT in the ln/exp
                # table set the softmax also uses (no sqrt-set reload) and
                # avoids an ACT->DVE reciprocal round-trip
                lnv = sm.tile([NH, 1], F32, tag=tag + "_sd")
                nc.scalar.activation(out=lnv, in_=mv[:, 1:2], func=AFT.Ln,
                                     bias=eps4)
                rstd = sm.tile([NH, 1], F32, tag=tag + "_rs")
                nc.scalar.activation(out=rstd, in_=lnv, func=AFT.Exp, scale=-0.5)
            else:
                std = sm.tile([NH, 1], F32, tag=tag + "_sd")
                nc.scalar.activation(out=std, in_=mv[:, 1:2], func=AFT.Sqrt,
                                     bias=eps4)
                rstd = sm.tile([NH, 1], F32, tag=tag + "_rs")
                nc.vector.reciprocal(out=rstd, in_=std)
            yield
            xn = sm.tile([NH, CH], F32, tag=tag + "_o")
            nc.vector.tensor_scalar(out=xn, in0=src, scalar1=mv[:, 0:1],
                                    scalar2=rstd, op0=ALU.subtract, op1=ALU.mult)
            nc.vector.tensor_mul(out=xn, in0=xn, in1=g_bc)
            nc.vector.tensor_add(out=xn, in0=xn, in1=b_bc)
            return xn

        def math_chain(b, xcs, sums4):
            # spatial mean: fold chunk sums, then fold the partition
            # quarters into pooled [4h, 8c] via selection matmul:
            # pooled[h, c] = sum_k hsel[k, h] * cmask[k, c] * sums[k]
            sums = sm.tile([P, 1], F32, tag="sums")
            nc.vector.reduce_sum(out=sums, in_=sums4, axis=AX.X)
            csums = sm.tile([P, CH], F32, tag="csums")
            nc.vector.tensor_scalar_mul(out=csums, in0=cmask, scalar1=sums)
            yield
            pooled_ps = ps.tile([NH, CH], F32, tag="ps")
            nc.tensor.matmul(pooled_ps, hsel, csums, start=True, stop=True)
            pooled = sm.tile([NH, CH], F32, tag="pooled")
            nc.vector.tensor_copy(out=pooled, in_=pooled_ps)
            yield
            xn = yield from layernorm(pooled, g1_bc, beta1_bc, "ln1")
            yield
            xnT = pe_t(xn, CH, "xnT")                    # [8, 4]
            yield
            qT = mm(wq_t, xnT, CH, NH, "qT")             # [8, 4] = Wq @ xn.T
            yield
            kT = mm(wk_t, xnT, CH, NH, "kT")
            yield
            v = mm(xnT, wv_t, NH, CH, "v")               # [4, 8] = xn @ Wv.T
            yield
            sc = mm(qT, kT, NH, NH)                      # psum [4h, 4g] = Q @ K.T
            es = sm.tile([NH, NH], F32, tag="es")
            nc.scalar.activation(out=es, in_=sc, func=AFT.Exp, scale=SCALE)
            yield
            rs = sm.tile([NH, 1], F32, tag="rs")
            nc.vector.reduce_sum(out=rs, in_=es, axis=AX.X)
            rr = sm.tile([NH, 1], F32, tag="rr")
            nc.vector.reciprocal(out=rr, in_=rs)
            yield
            attn = sm.tile([NH, NH], F32, tag="attn")
            nc.vector.tensor_scalar_mul(out=attn, in0=es, scalar1=rr)
            yield
            attnT = pe_t(attn, NH, "attnT")              # [4g, 4h]
            yield
            ao = mm(attnT, v, NH, CH, "ao")              # [4, 8] = attn @ V
            yield
            aoT = pe_t(ao, CH, "aoT")                    # [8, 4]
            yield
            o_ps = mm(aoT, wo_t, NH, CH)                 # psum [4, 8] = ao @ Wo.T
            xat = sm.tile([NH, CH], F32, tag="xat")
            nc.vector.tensor_add(out=xat, in0=o_ps, in1=bo_bc)
            nc.vector.tensor_add(out=xat, in0=xat, in1=pooled)
            yield
            xn2 = yield from layernorm(xat, g2_bc, beta2_bc, "ln2")
            yield
            xn2T = pe_t(xn2, CH, "xn2T")                 # [8, 4]
            yield
            h1_ps = mm(xn2T, w1_t, NH, HID)              # psum [4, 4] = xn2 @ W1.T
            h1b = sm.tile([NH, HID], F32, tag="h1b")
            nc.vector.tensor_add(out=h1b, in0=h1_ps, in1=b1_bc)
            yield
            h1g = sm.tile([NH, HID], F32, tag="h1g")
            nc.scalar.activation(out=h1g, in_=h1b, func=_GELU)
            yield
            h1gT = pe_t(h1g, HID, "h1gT")                # [4hid, 4h]
            yield
            f_ps = mm(h1gT, w2_t, NH, CH)                # psum [4, 8] = gelu @ W2.T
            xo = sm.tile([NH, CH], F32, tag="xo")
            nc.vector.tensor_add(out=xo, in0=f_ps, in1=b2_bc)
            nc.vector.tensor_add(out=xo, in0=xo, in1=xat)
            yield
            if _M4_FOLD:
                # m = 1 + aw = (g * x_out + 1) + (1 - g) * pooled
                d = sm.tile([NH, CH], F32, tag="d")
                nc.vector.tensor_scalar(out=d, in0=xo, scalar1=gsig4,
                                        scalar2=1.0, op0=ALU.mult, op1=ALU.add)
                m4 = sm.tile([NH, CH], F32, tag="m4")
                nc.vector.scalar_tensor_tensor(out=m4, in0=pooled, scalar=omg4,
                                               in1=d, op0=ALU.mult, op1=ALU.add)
            else:
                # m = 1 + aw = 1 + pooled + sigmoid(gate) * (x_out - pooled)
                d = sm.tile([NH, CH], F32, tag="d")
                nc.vector.tensor_sub(out=d, in0=xo, in1=pooled)
                m4 = sm.tile([NH, CH], F32, tag="m4")
                nc.vector.tensor_scalar(out=m4, in0=d, scalar1=gsig4,
                                        scalar2=1.0, op0=ALU.mult, op1=ALU.add)
                nc.vector.tensor_add(out=m4, in0=m4, in1=pooled)
            yield
            # expand m4 [4h, 8c] -> per-partition scalar mcol [128, 1] with
            # PE only: W128[h, k] = m4[h, c(k)]; mask rows by h(k); column
            # sums distribute the selected value to every partition k.
            m4T = pe_t(m4, CH, "m4T")                    # [8c, 4h]
            yield
            w128_ps = ps.tile([NH, P], F32, tag="ps")
            nc.tensor.matmul(w128_ps, m4T, b128, start=True, stop=True)
            v128 = sm.tile([NH, P], F32, tag="v128")
            nc.vector.tensor_mul(out=v128, in0=w128_ps, in1=ind128)
            yield
            mcol_ps = ps.tile([P, 1], F32, tag="ps")
            nc.tensor.matmul(mcol_ps, v128, ones4, start=True, stop=True)
            mcol = sm.tile([P, 1], F32, tag="mcol")
            nc.vector.tensor_copy(out=mcol, in_=mcol_ps)
            yield
            for c in range(NCHUNK):
                if _MULT_SPLIT and c % 2 == 1:
                    # odd chunks multiply on DVE so a batch's multiply phase
                    # runs on two engines at once (stores start sooner)
                    nc.vector.tensor_scalar_mul(out=xcs[c], in0=xcs[c],
                                                scalar1=mcol)
                else:
                    nc.scalar.activation(out=xcs[c], in_=xcs[c], func=AFT.Copy,
                                         scale=mcol)
                nc.scalar.dma_start(out=io["y"][b][:, c * (FREE // NCHUNK):(c + 1) * (FREE // NCHUNK)],
                                    in_=xcs[c])
                yield

        # Staggered software pipeline: batch b's chunked loads+reduces are
        # emitted just before its math chain joins; at most two math chains
        # are interleaved op-by-op (so the in-order engines always have a
        # ready op from the other chain), and batch b+2's loads are only
        # emitted after chain b fully completed (its stores free the SBUF
        # slots those loads need -- emitting earlier would deadlock).
        def start_batch(b):
            xcs = []
            sums4 = sm.tile([P, NCHUNK], F32, tag="sums4")
            for c in range(NCHUNK):
                xc = xp.tile([P, FREE // NCHUNK], F32, tag="xc")
                nc.sync.dma_start(out=xc,
                                  in_=io["x"][b][:, c * (FREE // NCHUNK):(c + 1) * (FREE // NCHUNK)])
                nc.vector.reduce_sum(out=sums4[:, c:c + 1], in_=xc, axis=AX.X)
                xcs.append(xc)
            return math_chain(b, xcs, sums4)

        def advance(active, g):
            try:
                next(g)
            except StopIteration:
                active.remove(g)

        def drive(active, until_remaining, head_start=0):
            # a younger chain's ops, placed in engine program order before an
            # older chain's, head-of-line block the engine while the younger
            # chain's inputs (its batch's loads) are still in flight -- so
            # give the older chain a solo head start before interleaving
            if not active:
                return
            oldest = active[0]
            for _ in range(head_start):
                if oldest not in active:
                    return
                advance(active, oldest)
            while len(active) > until_remaining:
                oldest = active[0]
                for g in list(active):
                    advance(active, g)
                if oldest not in active:
                    return

        active = []
        for b in range(BPC):
            active.append(start_batch(b))
            if len(active) == 2:
                drive(active, until_remaining=1, head_start=_HEAD_START)
        drive(active, until_remaining=0, head_start=_HEAD_START)


def _build():
    nc = bacc.Bacc()
    io = {}
    io["x"] = nc.declare_dram_parameter("x", [BPC, P, FREE], F32, isOutput=False)
    for name, shape in [
        ("wq_t", [CH, CH]), ("wk_t", [CH, CH]), ("wv_t", [CH, CH]),
        ("wo_t", [CH, CH]), ("w1_t", [CH, HID]), ("w2_t", [HID, CH]),
        ("bo", [CH]), ("b1", [HID]), ("b2", [CH]),
        ("g1", [CH]), ("beta1", [CH]), ("g2", [CH]), ("beta2", [CH]),
        ("gate", [1]), ("eye4", [NH, NH]),
        ("cmask", [P, CH]), ("hsel", [P, NH]),
        ("b128", [CH, P]), ("ind128", [NH, P]),
    ]:
        io[name] = nc.declare_dram_parameter(name, shape, F32, isOutput=False)
    io["y"] = nc.declare_dram_parameter("y", [BPC, P, FREE], F32, isOutput=True)
    with tile.TileContext(nc) as tc:
        _emit(nc, tc, io)
    nc.finalize()   # bacc lowering: splits multi-waits, act tables, etc.
    return nc


_NC_CACHE = {}


def _get_nc():
    key = (_RSTD_LNEXP, _HEAD_START, _M4_FOLD, NCHUNK, _XBUFS, _MULT_SPLIT)
    if key not in _NC_CACHE:
        _NC_CACHE[key] = _build()
    return _NC_CACHE[key]


def _prep_in_maps(inputs):
    x = np.ascontiguousarray(np.asarray(inputs["x"], dtype=np.float32))
    assert x.shape == (B, NH, CH, H, W), x.shape
    xr = x.reshape(NCORES, BPC, P, FREE)

    def t(a):
        return np.ascontiguousarray(np.asarray(a, dtype=np.float32).T)

    def v(a):
        return np.ascontiguousarray(np.asarray(a, dtype=np.float32))

    shared = {
        "wq_t": t(inputs["Wq"]), "wk_t": t(inputs["Wk"]), "wv_t": t(inputs["Wv"]),
        "wo_t": t(inputs["Wo"]), "w1_t": t(inputs["W1"]), "w2_t": t(inputs["W2"]),
        "bo": v(inputs["bo"]), "b1": v(inputs["b1"]), "b2": v(inputs["b2"]),
        "g1": v(inputs["g1"]), "beta1": v(inputs["beta1"]),
        "g2": v(inputs["g2"]), "beta2": v(inputs["beta2"]),
        "gate": v(inputs["gate"]),
        "eye4": np.eye(NH, dtype=np.float32),
    }
    k = np.arange(P)
    hk, ck = k // (CH * SPLIT), (k % (CH * SPLIT)) // SPLIT
    shared["cmask"] = ((ck[:, None] == np.arange(CH)[None, :]) / S).astype(np.float32)
    shared["hsel"] = (hk[:, None] == np.arange(NH)[None, :]).astype(np.float32)
    shared["b128"] = shared["cmask"].T.copy() * S
    shared["ind128"] = shared["hsel"].T.copy()
    return [dict(shared, x=xr[i]) for i in range(NCORES)]


def _run(inputs, **spmd_kwargs):
    from concourse.bass_utils import run_bass_kernel_spmd

    nc = _get_nc()
    in_maps = _prep_in_maps(inputs)
    res = run_bass_kernel_spmd(nc, in_maps, list(range(NCORES)), **spmd_kwargs)
    out = np.empty((B, NH, CH, H, W), dtype=np.float32)
    ov = out.reshape(NCORES, BPC, P, FREE)
    for i in range(NCORES):
        ov[i] = res.results[i]["y"]
    return out, res


def kernel(**inputs):
    return _run(inputs)[0]



# revision 20
# speedup vs baseline: 1.1764x; 1.1764x over previous
"""Trainium2 Bass kernel for nn_CrossHeadAttention.

Computation (per batch b):
  pooled = mean(x[b], spatial)                       # (NH, CH)
  aw     = tiny transformer block on pooled          # (NH, CH)
  out[b] = x[b] * (1 + aw)[..., None, None]

Memory-bound: 256 MiB in + 256 MiB out. Sharding: pure data-parallel over
batch (32 batches -> 8 cores x 4 batches). Per core, each batch's
(4, 8, 256, 256) slab is viewed as a [128, 16384] SBUF tile
(partition = head*32 + ch*4 + spatial_quarter), streamed in 1 MiB chunks.

Engine plan (v1 redesign):
  Sync   : all x chunk loads (HWDGE, one FIFO ring).
  ACT    : per-chunk spatial reduce via activation(Copy, accum_out=sums),
           softmax Exp, gelu Tanh, and a minority of the output multiplies.
           Only {Copy, Exp, Tanh} are used -> all live in act table set 0
           ("exp_and_others"), so at most one table load ever happens.
  DVE    : the tiny per-batch math chain (LN via Newton-rsqrt with the
           quake bit-trick seed, softmax normalize folded post-matmul,
           tanh-gelu), plus most output multiplies.
  PE     : 9 small matmuls per batch; transposed-output tricks remove all
           but two explicit transposes. Wo is folded into Wv on the host
           ((attn @ V) @ Wo.T == attn @ (V @ Wo.T)), SCALE into Wq, bo into
           the attention value matrix via the es row-sum, sigmoid(gate) is
           computed on the host.
  GpSimd : all y chunk stores (SWDGE) - keeps store issue off ACT.

All small constants are packed into ONE [128, NCOL] DRAM tensor loaded
with a single HWDGE DMA at program start (the baseline's 15 serial SWDGE
const loads cost ~20us of startup latency).

Emission scheduler: loader(b) (loads + ACT reduces) is pumped at a 1:R
ratio against worker(b-1)'s chain so the younger batch's ACT reduces
don't head-of-line block the older chain's Exp/Tanh, and worker(b+1)'s
chain is interleaved 1:1 with worker(b)'s multiply/store phase.
"""

from contextlib import ExitStack

import numpy as np

import concourse.bacc as bacc
import concourse.bass as bass
import concourse.tile as tile
from concourse import mybir

NCORES = 8
B, NH, CH = 32, 4, 8
H = W = 256
S = H * W                  # spatial elements per (b, h, c) plane
HID = 4
BPC = B // NCORES          # batches per core
P = 128                    # SBUF partitions
SPLIT = P // (NH * CH)     # spatial quarters mapped to partitions
FREE = S // SPLIT          # free-dim elements per partition
NCHUNK = 8
CHW = FREE // NCHUNK       # chunk width (free elems)
SCALE = CH ** -0.5
EPS = 1e-5
F32 = mybir.dt.float32
U32 = mybir.dt.uint32
AFT = mybir.ActivationFunctionType
ALU = mybir.AluOpType
AX = mybir.AxisListType

_LRATIO = 2        # pump 1 loader step per _LRATIO worker steps
_RED_ACT = True    # chunk reduces on ACT via accum_out (else DVE tensor_reduce)
_USE_TTR = False   # tensor_tensor_reduce wedges the exec unit on this runtime
_ACT_MULTS = (0, 4)   # chunk indices whose multiply runs on ACT (rest DVE)
_STORE_GPSIMD = True  # stores via SWDGE (else ACT/HWDGE)
_NEWTON_ITERS = 2
_XBUFS = 24

# --- packed constant-table column layout (built in _prep_in_maps) -------
_COLS = {}


def _col(name, parts, width, cur=[0]):
    _COLS[name] = (parts, cur[0], width)
    cur[0] += width
    return _COLS[name]


_col("wqs", 8, 8)       # Wq.T * SCALE
_col("wkt", 8, 8)       # Wk.T
_col("wvo", 8, 8)       # (Wo @ Wv).T
_col("w1t", 8, 4)       # W1.T
_col("w2t", 4, 8)       # W2.T
_col("eye4", 4, 4)
_col("hselT", 4, 128)   # [h, k] = (h(k)==h)
_col("hsel", 128, 4)    # [k, h] = (h(k)==h)
_col("cm9", 128, 9)     # [k, c] = (c(k)==c)/S ; col 8 = 1/(8S)
_col("cm01", 128, 8)    # [k, c] = (c(k)==c)
_col("g1c", 8, 1)
_col("be1c", 8, 1)
_col("g2c", 8, 1)
_col("be2c", 8, 1)
_col("b1h", 4, 1)
_col("b2bc", 4, 8)      # b2 broadcast over rows
_col("bobc", 4, 8)      # bo broadcast over rows
_col("gsig", 4, 1)      # sigmoid(gate)
_col("omg", 4, 1)       # 1 - sigmoid(gate)
NCOL = max(o + w for _, o, w in _COLS.values())


def _emit(nc, tc, io):
    with ExitStack() as ctx:
        cp = ctx.enter_context(tc.tile_pool(name="cp", bufs=1))
        xp = ctx.enter_context(tc.tile_pool(name="xp", bufs=_XBUFS))
        dp = ctx.enter_context(tc.tile_pool(name="dp", bufs=1))
        sm = ctx.enter_context(tc.tile_pool(name="sm", bufs=4))
        ps = ctx.enter_context(tc.tile_pool(name="ps", bufs=8, space="PSUM"))

        CT = cp.tile([P, NCOL], F32, tag="ct")
        nc.sync.dma_start(out=CT, in_=io["ct"][:])

        def C(name):
            parts, off, width = _COLS[name]
            return CT[0:parts, off:off + width]

        wqs, wkt = C("wqs"), C("wkt")
        wvo, w1t, w2t, eye4 = C("wvo"), C("w1t"), C("w2t"), C("eye4")
        hselT, hsel, cm9, cm01 = C("hselT"), C("hsel"), C("cm9"), C("cm01")
        g1c, be1c, g2c, be2c = C("g1c"), C("be1c"), C("g2c"), C("be2c")
        b1h, b2bc, bobc = C("b1h"), C("b2bc"), C("bobc")
        gsig, omg = C("gsig"), C("omg")

        dummy = dp.tile([P, CHW], F32, tag="dummy")   # ACT reduce main out

        state = {}   # b -> (sums4, xcs, mcol-slot dict)

        def loader(b):
            sums4 = sm.tile([P, NCHUNK], F32, tag="sums4")
            xcs = []
            for c in range(NCHUNK):
                xc = xp.tile([P, CHW], F32, tag="xc")
                nc.sync.dma_start(out=xc,
                                  in_=io["x"][b][:, c * CHW:(c + 1) * CHW])
                if _RED_ACT:
                    nc.scalar.activation(out=dummy, in_=xc, func=AFT.Copy,
                                         accum_out=sums4[:, c:c + 1])
                else:
                    nc.vector.reduce_sum(out=sums4[:, c:c + 1], in_=xc,
                                         axis=AX.X)
                xcs.append(xc)
                yield "load"
            state[b] = (sums4, xcs)

        def rsqrt_act(v, tag):
            # rstd = exp(-0.5 * ln(v)); Ln and Exp share act table set 6
            # ("natural_log_exp_and_others") with Copy, so no table switches.
            lnv = sm.tile([NH, 1], F32, tag=tag + "l")
            nc.scalar.activation(out=lnv, in_=v, func=AFT.Ln)
            y = sm.tile([NH, 1], F32, tag=tag + "y")
            nc.scalar.activation(out=y, in_=lnv, func=AFT.Exp, scale=-0.5)
            return y

        def worker(b):
            sums4, xcs = state[b]
            voext = sm.tile([NH, 9], F32, tag="voext")
            nc.vector.memset(voext[:, 8:9], 1.0)
            # --- pooling ---
            q = sm.tile([P, 1], F32, tag="q")
            nc.vector.reduce_sum(out=q, in_=sums4, axis=AX.X)
            cs9 = sm.tile([P, 9], F32, tag="cs9")
            nc.vector.tensor_scalar_mul(out=cs9, in0=cm9, scalar1=q)
            yield "chain"
            p9ps = ps.tile([NH, 9], F32, tag="ps")
            nc.tensor.matmul(p9ps, hsel, cs9, start=True, stop=True)
            p9 = sm.tile([NH, 9], F32, tag="p9")
            nc.vector.tensor_copy(out=p9, in_=p9ps)
            pooled, mu = p9[:, 0:8], p9[:, 8:9]
            yield "chain"
            # --- LN1 ---
            cent = sm.tile([NH, CH], F32, tag="cent")
            nc.vector.tensor_scalar_sub(out=cent, in0=pooled, scalar1=mu)
            d8 = sm.tile([NH, CH], F32, tag="d8")
            v1 = sm.tile([NH, 1], F32, tag="v1")
            if _USE_TTR:
                nc.vector.tensor_tensor_reduce(out=d8, in0=cent, in1=cent,
                                               scale=0.125, scalar=EPS,
                                               op0=ALU.mult, op1=ALU.add,
                                               accum_out=v1)
            else:
                nc.vector.tensor_mul(out=d8, in0=cent, in1=cent)
                nc.vector.reduce_sum(out=v1, in_=d8, axis=AX.X)
                nc.vector.tensor_scalar(out=v1, in0=v1, scalar1=0.125,
                                        scalar2=EPS, op0=ALU.mult, op1=ALU.add)
            yield "chain"
            rstd1 = rsqrt_act(v1, "r1")
            xnr = sm.tile([NH, CH], F32, tag="xnr")
            nc.vector.tensor_scalar_mul(out=xnr, in0=cent, scalar1=rstd1)
            yield "chain"
            xTps = ps.tile([CH, NH], F32, tag="ps")
            nc.tensor.transpose(xTps, xnr, eye4)
            xnT = sm.tile([CH, NH], F32, tag="xnT")
            nc.vector.tensor_scalar(out=xnT, in0=xTps, scalar1=g1c,
                                    scalar2=be1c, op0=ALU.mult, op1=ALU.add)
            yield "chain"
            # --- attention ---
            qps = ps.tile([CH, NH], F32, tag="ps")
            nc.tensor.matmul(qps, wqs, xnT, start=True, stop=True)
            qT = sm.tile([CH, NH], F32, tag="qT")
            nc.vector.tensor_copy(out=qT, in_=qps)
            kps = ps.tile([CH, NH], F32, tag="ps")
            nc.tensor.matmul(kps, wkt, xnT, start=True, stop=True)
            kT = sm.tile([CH, NH], F32, tag="kT")
            nc.vector.tensor_copy(out=kT, in_=kps)
            yield "chain"
            vops = ps.tile([NH, CH], F32, tag="ps")
            nc.tensor.matmul(vops, xnT, wvo, start=True, stop=True)
            nc.vector.tensor_add(out=voext[:, 0:8], in0=vops, in1=bobc)
            yield "chain"
            scps = ps.tile([NH, NH], F32, tag="ps")
            nc.tensor.matmul(scps, kT, qT, start=True, stop=True)
            esT = sm.tile([NH, NH], F32, tag="esT")
            nc.scalar.activation(out=esT, in_=scps, func=AFT.Exp)
            yield "chain"
            aops = ps.tile([NH, 9], F32, tag="ps")
            nc.tensor.matmul(aops, esT, voext, start=True, stop=True)
            rr = sm.tile([NH, 1], F32, tag="rr")
            nc.vector.reciprocal(out=rr, in_=aops[:, 8:9])
            xat = sm.tile([NH, CH], F32, tag="xat")
            nc.vector.scalar_tensor_tensor(out=xat, in0=aops[:, 0:8],
                                           scalar=rr, in1=pooled,
                                           op0=ALU.mult, op1=ALU.add)
            b2x = sm.tile([NH, CH], F32, tag="b2x")
            nc.vector.tensor_add(out=b2x, in0=xat, in1=b2bc)
            yield "chain"
            # --- LN2 ---
            s2 = sm.tile([NH, 1], F32, tag="s2")
            nc.vector.reduce_sum(out=s2, in_=xat, axis=AX.X)
            mu2 = sm.tile([NH, 1], F32, tag="mu2")
            nc.vector.tensor_scalar_mul(out=mu2, in0=s2, scalar1=0.125)
            cent2 = sm.tile([NH, CH], F32, tag="cent2")
            nc.vector.tensor_scalar_sub(out=cent2, in0=xat, scalar1=mu2)
            d8b = sm.tile([NH, CH], F32, tag="d8b")
            v2 = sm.tile([NH, 1], F32, tag="v2")
            if _USE_TTR:
                nc.vector.tensor_tensor_reduce(out=d8b, in0=cent2, in1=cent2,
                                               scale=0.125, scalar=EPS,
                                               op0=ALU.mult, op1=ALU.add,
                                               accum_out=v2)
            else:
                nc.vector.tensor_mul(out=d8b, in0=cent2, in1=cent2)
                nc.vector.reduce_sum(out=v2, in_=d8b, axis=AX.X)
                nc.vector.tensor_scalar(out=v2, in0=v2, scalar1=0.125,
                                        scalar2=EPS, op0=ALU.mult, op1=ALU.add)
            yield "chain"
            rstd2 = rsqrt_act(v2, "r2")
            xn2r = sm.tile([NH, CH], F32, tag="xn2r")
            nc.vector.tensor_scalar_mul(out=xn2r, in0=cent2, scalar1=rstd2)
            yield "chain"
            x2Tps = ps.tile([CH, NH], F32, tag="ps")
            nc.tensor.transpose(x2Tps, xn2r, eye4)
            xn2T = sm.tile([CH, NH], F32, tag="xn2T")
            nc.vector.tensor_scalar(out=xn2T, in0=x2Tps, scalar1=g2c,
                                    scalar2=be2c, op0=ALU.mult, op1=ALU.add)
            yield "chain"
            # --- FFN (tanh-gelu; |err| <= ~1e-3, well inside tolerance) ---
            h1ps = ps.tile([HID, NH], F32, tag="ps")
            nc.tensor.matmul(h1ps, w1t, xn2T, start=True, stop=True)
            h1bT = sm.tile([HID, NH], F32, tag="h1bT")
            nc.vector.tensor_scalar(out=h1bT, in0=h1ps, scalar1=b1h,
                                    scalar2=None, op0=ALU.add)
            sq = sm.tile([HID, NH], F32, tag="sq")
            nc.vector.tensor_mul(out=sq, in0=h1bT, in1=h1bT)
            nc.vector.tensor_scalar(out=sq, in0=sq, scalar1=0.044715,
                                    scalar2=1.0, op0=ALU.mult, op1=ALU.add)
            nc.vector.tensor_mul(out=sq, in0=sq, in1=h1bT)
            yield "chain"
            # gelu_tanh(x) = x * sigmoid(2*0.79788456*(x + 0.044715 x^3))
            ew = sm.tile([HID, NH], F32, tag="ew")
            nc.scalar.activation(out=ew, in_=sq, func=AFT.Exp,
                                 scale=-1.5957691216057308)
            yield "chain"
            gel = sm.tile([HID, NH], F32, tag="gel")
            nc.vector.tensor_scalar(out=ew, in0=ew, scalar1=1.0, scalar2=None,
                                    op0=ALU.add)
            nc.vector.reciprocal(out=gel, in_=ew)
            nc.vector.tensor_mul(out=gel, in0=gel, in1=h1bT)
            yield "chain"
            fps = ps.tile([NH, CH], F32, tag="ps")
            nc.tensor.matmul(fps, gel, w2t, start=True, stop=True)
            xo = sm.tile([NH, CH], F32, tag="xo")
            nc.vector.tensor_add(out=xo, in0=fps, in1=b2x)
            # --- gate ---
            dd = sm.tile([NH, CH], F32, tag="dd")
            nc.vector.tensor_scalar(out=dd, in0=xo, scalar1=gsig, scalar2=1.0,
                                    op0=ALU.mult, op1=ALU.add)
            m4 = sm.tile([NH, CH], F32, tag="m4")
            nc.vector.scalar_tensor_tensor(out=m4, in0=pooled, scalar=omg,
                                           in1=dd, op0=ALU.mult, op1=ALU.add)
            yield "chain"
            # --- expand m4 -> per-partition scalar mcol [128, 1] ---
            Ups = ps.tile([P, CH], F32, tag="ps")
            nc.tensor.matmul(Ups, hselT, m4, start=True, stop=True)
            d128 = sm.tile([P, CH], F32, tag="d128")
            mcol = sm.tile([P, 1], F32, tag="mcol")
            if _USE_TTR:
                nc.vector.tensor_tensor_reduce(out=d128, in0=Ups, in1=cm01,
                                               scale=1.0, scalar=0.0,
                                               op0=ALU.mult, op1=ALU.add,
                                               accum_out=mcol)
            else:
                nc.vector.tensor_mul(out=d128, in0=Ups, in1=cm01)
                nc.vector.reduce_sum(out=mcol, in_=d128, axis=AX.X)
            yield "chain"
            # --- multiply + store ---
            for c in range(NCHUNK):
                if c in _ACT_MULTS:
                    nc.scalar.activation(out=xcs[c], in_=xcs[c], func=AFT.Copy,
                                         scale=mcol)
                else:
                    nc.vector.tensor_scalar_mul(out=xcs[c], in0=xcs[c],
                                                scalar1=mcol)
                eng = nc.gpsimd if _STORE_GPSIMD else nc.scalar
                eng.dma_start(out=io["y"][b][:, c * CHW:(c + 1) * CHW],
                              in_=xcs[c])
                yield "mult"
            del state[b]

        # ---------------- emission scheduler ----------------
        loaders = [loader(b) for b in range(BPC)]
        workers = [worker(b) for b in range(BPC)]
        ldone = [False] * BPC
        phase = {}

        def step(g):
            try:
                phase[g] = next(g)
                return True
            except StopIteration:
                return False

        def drain_loader(b):
            if not ldone[b]:
                while step(loaders[b]):
                    pass
                ldone[b] = True

        drain_loader(0)
        active = [workers[0]]
        wn = 1       # next worker to start
        lp = 1       # loader being pumped incrementally
        tick = 0
        while active:
            tick += 1
            g0 = active[0]
            if not step(g0):
                active.pop(0)
            elif phase[g0] == "mult" and len(active) == 1 and wn < BPC:
                drain_loader(wn)
                lp = wn + 1
                active.append(workers[wn])
                wn += 1
            if len(active) > 1 and not step(active[1]):
                active.pop(1)
            if tick % _LRATIO == 0 and lp < BPC and not ldone[lp]:
                if not step(loaders[lp]):
                    ldone[lp] = True
                    lp += 1
        for b in range(BPC):
            drain_loader(b)


def _build():
    nc = bacc.Bacc()
    io = {}
    io["x"] = nc.declare_dram_parameter("x", [BPC, P, FREE], F32, isOutput=False)
    io["ct"] = nc.declare_dram_parameter("ct", [P, NCOL], F32, isOutput=False)
    io["y"] = nc.declare_dram_parameter("y", [BPC, P, FREE], F32, isOutput=True)
    with tile.TileContext(nc) as tc:
        _emit(nc, tc, io)
    nc.finalize()
    return nc


_NC_CACHE = {}


def _get_nc():
    key = (_LRATIO, _ACT_MULTS, _STORE_GPSIMD, _NEWTON_ITERS, _XBUFS,
           _RED_ACT, _USE_TTR)
    if key not in _NC_CACHE:
        _NC_CACHE[key] = _build()
    return _NC_CACHE[key]


def _build_ct(inputs):
    ct = np.zeros((P, NCOL), dtype=np.float32)

    def put(name, arr):
        parts, off, width = _COLS[name]
        a = np.asarray(arr, dtype=np.float32)
        assert a.shape == (parts, width), (name, a.shape)
        ct[0:parts, off:off + width] = a

    Wq = np.asarray(inputs["Wq"], dtype=np.float32)
    Wk = np.asarray(inputs["Wk"], dtype=np.float32)
    Wv = np.asarray(inputs["Wv"], dtype=np.float32)
    Wo = np.asarray(inputs["Wo"], dtype=np.float32)
    W1 = np.asarray(inputs["W1"], dtype=np.float32)
    W2 = np.asarray(inputs["W2"], dtype=np.float32)
    put("wqs", (Wq * SCALE).T)
    put("wkt", Wk.T)
    put("wvo", (Wo @ Wv).T)
    put("w1t", W1.T)
    put("w2t", W2.T)
    put("eye4", np.eye(NH))
    k = np.arange(P)
    hk, ck = k // (CH * SPLIT), (k % (CH * SPLIT)) // SPLIT
    hsel = (hk[:, None] == np.arange(NH)[None, :]).astype(np.float32)
    put("hselT", hsel.T)
    put("hsel", hsel)
    cm9 = np.zeros((P, 9), dtype=np.float32)
    cm9[:, 0:8] = (ck[:, None] == np.arange(CH)[None, :]) / S
    cm9[:, 8] = 1.0 / (8.0 * S)
    put("cm9", cm9)
    put("cm01", (ck[:, None] == np.arange(CH)[None, :]).astype(np.float32))
    put("g1c", np.asarray(inputs["g1"], np.float32)[:, None])
    put("be1c", np.asarray(inputs["beta1"], np.float32)[:, None])
    put("g2c", np.asarray(inputs["g2"], np.float32)[:, None])
    put("be2c", np.asarray(inputs["beta2"], np.float32)[:, None])
    put("b1h", np.asarray(inputs["b1"], np.float32)[:, None])
    put("b2bc", np.tile(np.asarray(inputs["b2"], np.float32)[None, :], (NH, 1)))
    put("bobc", np.tile(np.asarray(inputs["bo"], np.float32)[None, :], (NH, 1)))
    gate = float(np.asarray(inputs["gate"], np.float64).reshape(-1)[0])
    gs = 1.0 / (1.0 + np.exp(-gate))
    put("gsig", np.full((NH, 1), gs))
    put("omg", np.full((NH, 1), 1.0 - gs))
    return ct


def _prep_in_maps(inputs):
    x = np.ascontiguousarray(np.asarray(inputs["x"], dtype=np.float32))
    assert x.shape == (B, NH, CH, H, W), x.shape
    xr = x.reshape(NCORES, BPC, P, FREE)
    ct = _build_ct(inputs)
    return [{"x": xr[i], "ct": ct} for i in range(NCORES)]


def _run(inputs, **spmd_kwargs):
    from concourse.bass_utils import run_bass_kernel_spmd

    nc = _get_nc()
    in_maps = _prep_in_maps(inputs)
    res = run_bass_kernel_spmd(nc, in_maps, list(range(NCORES)), **spmd_kwargs)
    out = np.empty((B, NH, CH, H, W), dtype=np.float32)
    ov = out.reshape(NCORES, BPC, P, FREE)
    for i in range(NCORES):
        ov[i] = res.results[i]["y"]
    return out, res


def kernel(**inputs):
    return _run(inputs)[0]
